# revision 9
# baseline (speedup 1.0000x reference)
"""Bass/Tile TRN2 kernel for nn_MultiHeadAttention_11330123727139.

Reference computation (full shapes):
  Q [1,1024], K [2048,1024], V [2048,128,128,3],
  WQ/WK [8,1024,1024], bQ/bK [8,1024]
  q = Q @ WQ[h].T + bQ[h]; k = K @ WK[h].T + bK[h]
  logits[h,m] = (q[h] . k[h,m]) / 1024
  w = softmax(logits, axis=m);  out[h] = sum_m w[h,m] * V[m]

Algebra (exact): q.k = q^T WK K[m] + q.bK; the bK term is constant in m
so it cancels in softmax -> bK unused. With t[h] = WK[h]^T q[h]:
logits[h,m] = t[h].K[m]/1024. The 34-GFLOP K-projection disappears;
the kernel is memory-bound on V.

logits/1024 ~ O(0.05) so softmax needs no max subtraction. We compute
unnormalized u = exp(l/1024), the weighted V sum with u, and carry
sum(u) inside the ReduceScatter payload; division happens after the
collective. Only 2 collectives: AllGather(t), ReduceScatter(partials).

Precision: V / W-matrices / K cast to bf16 on the host (half DMA,
1-pass bf16 matmul); fp32 accumulation everywhere. Simulated
end-to-end relative error: 2.4e-3.

Sharding (8 cores): core c owns head c's WQ/WK and K/V rows
[256c, 256c+256). ReduceScatter leaves head c's finished image on
core c; the host stacks the 8 images.
"""

import numpy as np

D = 1024
H = 8
M = 2048
NCORES = 8
MS = M // NCORES          # 256 neighbors per core
PIX = 128 * 128 * 3       # 49152 pixels per image
CH = 4096                 # V free-dim chunk per DMA tile (bf16 -> 1MB)
RSW = PIX + 8             # RS row: 49152 pixels + sumexp + 7 pad (32B align)


def _build_nc():
    import concourse.bacc as bacc
    import concourse.mybir as mybir
    import concourse.tile as tile
    from concourse.masks import make_identity

    fp32 = mybir.dt.float32
    bf16 = mybir.dt.bfloat16
    ALU = mybir.AluOpType
    AX = mybir.AxisListType
    ACT = mybir.ActivationFunctionType

    nc = bacc.Bacc("TRN2", target_bir_lowering=False, debug=False,
                   num_devices=NCORES)

    q0 = nc.dram_tensor("q0", [1, D], bf16, kind="ExternalInput")
    wq = nc.dram_tensor("wq", [D, D], bf16, kind="ExternalInput")
    bq = nc.dram_tensor("bq", [128, 8], fp32, kind="ExternalInput")
    wk = nc.dram_tensor("wk", [D, D], bf16, kind="ExternalInput")
    kt = nc.dram_tensor("kt", [D, MS], bf16, kind="ExternalInput")
    v = nc.dram_tensor("v", [MS, PIX], bf16, kind="ExternalInput")
    out = nc.dram_tensor("out", [PIX], fp32, kind="ExternalOutput")

    RG = [list(range(NCORES))]

    with tile.TileContext(nc) as tc:
        with (
            tc.tile_pool(name="wts", bufs=1) as wts,
            tc.tile_pool(name="sm", bufs=1) as sm,
            tc.tile_pool(name="scr", bufs=2) as scr,
            tc.tile_pool(name="vp", bufs=6) as vp,
            tc.tile_pool(name="stg", bufs=3) as stg,
            tc.tile_pool(name="ps8", bufs=8, space="PSUM") as ps8,
            tc.tile_pool(name="dram", bufs=1, space="DRAM") as dram,
        ):
            ident = sm.tile([128, 128], fp32)
            make_identity(nc, ident)

            # Dummy collective with no data deps: absorbs the one-time
            # runtime collective-entry barrier + ncfw warmup during the
            # setup phase, so AllGather(t) later pays only mesh latency.
            warm_in = dram.tile([8], fp32)
            warm_out = dram.tile([64], fp32, addr_space="Shared")
            nc.gpsimd.collective_compute(
                "AllGather", ALU.bypass, replica_groups=RG,
                ins=[warm_in[:]], outs=[warm_out[:]])

            # ---- load inputs ----
            qb = wts.tile([128, D], bf16)
            nc.sync.dma_start(qb[:], q0[0:1, :].to_broadcast([128, D]))

            # wq/wk gate t (and thus the AllGather) -> sync queue, first.
            # kt is only needed post-AG -> scalar queue, ahead of v1 loads.
            wq_sb = wts.tile([128, 8 * D], bf16)
            wk_sb = wts.tile([128, 8 * D], bf16)
            kt_sb = wts.tile([128, 8 * MS], bf16)
            for i in range(8):
                nc.sync.dma_start(wq_sb[:, i * D:(i + 1) * D],
                                  wq[i * 128:(i + 1) * 128, :])
                nc.sync.dma_start(wk_sb[:, i * D:(i + 1) * D],
                                  wk[i * 128:(i + 1) * 128, :])
                nc.scalar.dma_start(kt_sb[:, i * MS:(i + 1) * MS],
                                    kt[i * 128:(i + 1) * 128, :])
            bq_sb = sm.tile([128, 8], fp32)
            nc.sync.dma_start(bq_sb[:], bq[:, :])

            # ---- q[e] = sum_d WQ[e,d]*Q[d] + bQ[e] ----
            q_cols = sm.tile([128, 8], fp32)
            for i in range(8):
                sc = scr.tile([128, D], bf16, tag="ttr", name=f"sc{i}")
                nc.vector.tensor_tensor(out=sc[:],
                                        in0=wq_sb[:, i * D:(i + 1) * D],
                                        in1=qb[:], op=ALU.mult)
                nc.vector.tensor_reduce(out=q_cols[:, i:i + 1], in_=sc[:],
                                        axis=AX.X, op=ALU.add)
            nc.vector.tensor_tensor(out=q_cols[:], in0=q_cols[:],
                                    in1=bq_sb[:], op=ALU.add)
            q_colsb = sm.tile([128, 8], bf16)
            nc.vector.tensor_copy(out=q_colsb[:], in_=q_cols[:])

            # ---- t[d] = sum_e WK[e,d] q[e] -> t_row [1,1024] fp32 ----
            t_row = sm.tile([1, D], fp32)
            for j in range(2):
                t_ps = ps8.tile([1, 512], fp32, tag="ps", name=f"tps{j}")
                for i in range(8):
                    nc.tensor.matmul(
                        t_ps[:],
                        lhsT=q_colsb[:, i:i + 1],
                        rhs=wk_sb[:, i * D + j * 512: i * D + (j + 1) * 512],
                        start=(i == 0),
                        stop=(i == 7),
                    )
                nc.vector.tensor_copy(out=t_row[0:1, j * 512:(j + 1) * 512],
                                      in_=t_ps[:])

            # ---- AllGather t (the only pre-V collective) ----
            t_loc = dram.tile([D], fp32)
            nc.sync.dma_start(t_loc[:], t_row[0:1, :])
            t_all = dram.tile([H, D], fp32, addr_space="Shared")
            nc.gpsimd.collective_compute(
                "AllGather", ALU.bypass, replica_groups=RG,
                ins=[t_loc[:]], outs=[t_all[:]])

            # t_all -> bf16 stationary tiles [128 d, 8 h] per d-tile
            t_nat = sm.tile([8, D], fp32)
            nc.sync.dma_start(t_nat[:], t_all[:])
            t_stat = sm.tile([128, 64], bf16)
            for j in range(8):
                tp = ps8.tile([128, 8], fp32, tag="ps", name=f"ttp{j}")
                nc.tensor.transpose(tp[:], t_nat[:, j * 128:(j + 1) * 128],
                                    ident[0:8, 0:8])
                nc.vector.tensor_copy(out=t_stat[:, j * 8:(j + 1) * 8],
                                      in_=tp[:])

            # ---- local logits [8 heads, 256 m] (fp32 accum) ----
            lg_ps = ps8.tile([8, MS], fp32, tag="ps", name="lgps")
            for j in range(8):
                nc.tensor.matmul(
                    lg_ps[:],
                    lhsT=t_stat[:, j * 8:(j + 1) * 8],
                    rhs=kt_sb[:, j * MS:(j + 1) * MS],
                    start=(j == 0),
                    stop=(j == 7),
                )
            lg_loc_sb = sm.tile([8, MS], fp32)
            nc.vector.tensor_copy(out=lg_loc_sb[:], in_=lg_ps[:])

            # ---- unnormalized weights u = exp(l/1024), se = sum(u) ----
            w_exp = sm.tile([8, MS], fp32)
            se = sm.tile([8, 1], fp32)
            nc.scalar.activation(out=w_exp[:], in_=lg_loc_sb[:], func=ACT.Exp,
                                 scale=1.0 / float(D), accum_out=se[:])

            wvb = sm.tile([128, 16], bf16)
            for k in range(2):
                wp = ps8.tile([128, 8], fp32, tag="ps", name=f"wp{k}")
                nc.tensor.transpose(wp[:], w_exp[:, k * 128:(k + 1) * 128],
                                    ident[0:8, 0:8])
                nc.vector.tensor_copy(out=wvb[:, k * 8:(k + 1) * 8], in_=wp[:])

            # ---- streamed weighted sum over V shard (bf16 x bf16 -> fp32).
            # Two ReduceScatters so the first overlaps the V-loop tail;
            # sumexp rides in payload A (slot PIXH) so half-A normalization
            # can start as soon as RS#1 lands. ----
            PIXH = PIX // 2
            NBLK = PIX // CH
            rs_in_a = dram.tile([H, PIXH + 8], fp32)
            rs_in_b = dram.tile([H, PIXH], fp32)
            nc.sync.dma_start(rs_in_a[:, PIXH:PIXH + 1], se[:])
            zpad = sm.tile([8, 7], fp32)
            nc.vector.memset(zpad[:], 0.0)
            nc.sync.dma_start(rs_in_a[:, PIXH + 1:PIXH + 8], zpad[:])

            rs_out_a = dram.tile([PIXH + 8], fp32)
            rs_out_b = dram.tile([PIXH], fp32)

            for blk in range(NBLK):
                v0 = vp.tile([128, CH], bf16, tag="v0", name=f"v0_{blk}")
                v1 = vp.tile([128, CH], bf16, tag="v1", name=f"v1_{blk}")
                nc.sync.dma_start(v0[:], v[0:128, blk * CH:(blk + 1) * CH])
                nc.scalar.dma_start(v1[:], v[128:256, blk * CH:(blk + 1) * CH])
                for g in range(CH // 2048):
                    pss = []
                    for s in range(4):
                        ps_t = ps8.tile([8, 512], fp32, tag="ps",
                                        name=f"ps{blk}_{g}_{s}")
                        pss.append(ps_t)
                    for s in range(4):
                        o = g * 2048 + s * 512
                        nc.tensor.matmul(pss[s][:], lhsT=wvb[:, 0:8],
                                         rhs=v0[:, o:o + 512],
                                         start=True, stop=False)
                    for s in range(4):
                        o = g * 2048 + s * 512
                        nc.tensor.matmul(pss[s][:], lhsT=wvb[:, 8:16],
                                         rhs=v1[:, o:o + 512],
                                         start=False, stop=True)
                    st = stg.tile([8, 2048], fp32, tag="st",
                                  name=f"st{blk}_{g}")
                    for s in range(4):
                        if s % 2 == 0:
                            nc.scalar.copy(out=st[:, s * 512:(s + 1) * 512],
                                           in_=pss[s][:])
                        else:
                            nc.vector.tensor_copy(
                                out=st[:, s * 512:(s + 1) * 512],
                                in_=pss[s][:])
                    o = blk * CH + g * 2048
                    if o < PIXH:
                        nc.gpsimd.dma_start(rs_in_a[:, o:o + 2048], st[:])
                    else:
                        nc.gpsimd.dma_start(rs_in_b[:, o - PIXH:o - PIXH + 2048],
                                            st[:])
                if blk == NBLK // 2 - 1:
                    nc.gpsimd.collective_compute(
                        "ReduceScatter", ALU.add, replica_groups=RG,
                        ins=[rs_in_a[:]], outs=[rs_out_a[:]])

            nc.gpsimd.collective_compute(
                "ReduceScatter", ALU.add, replica_groups=RG,
                ins=[rs_in_b[:]], outs=[rs_out_b[:]])

            # normalize: out = rs_out / sum(u); half A overlaps RS#2
            seb = sm.tile([128, 1], fp32)
            nc.sync.dma_start(seb[:],
                              rs_out_a[PIXH:PIXH + 1].to_broadcast([128, 1]))
            inv = sm.tile([128, 1], fp32)
            nc.vector.reciprocal(inv[:], seb[:])
            ob_a = sm.tile([128, PIXH // 128], fp32)
            nc.sync.dma_start(ob_a[:], rs_out_a[0:PIXH])
            nc.vector.tensor_scalar_mul(ob_a[:], ob_a[:], inv[:])
            nc.sync.dma_start(out[0:PIXH], ob_a[:])
            ob_b = sm.tile([128, PIXH // 128], fp32)
            nc.scalar.dma_start(ob_b[:], rs_out_b[0:PIXH])
            nc.vector.tensor_scalar_mul(ob_b[:], ob_b[:], inv[:])
            nc.scalar.dma_start(out[PIXH:PIX], ob_b[:])

    nc.compile()
    return nc


_NC_CACHE = []


def kernel(Q, K, V, WQ, bQ, WK, bK):
    import ml_dtypes
    from concourse.bass_utils import run_bass_kernel_spmd

    if not _NC_CACHE:
        _NC_CACHE.append(_build_nc())
    nc = _NC_CACHE[0]

    bf16 = ml_dtypes.bfloat16
    Q = np.asarray(Q, dtype=np.float32)
    K = np.asarray(K, dtype=np.float32)
    V = np.asarray(V, dtype=np.float32)
    WQ = np.asarray(WQ, dtype=np.float32)
    bQ = np.asarray(bQ, dtype=np.float32)
    WK = np.asarray(WK, dtype=np.float32)

    q0b = np.ascontiguousarray(Q.reshape(1, D)).astype(bf16)
    in_maps = []
    for c in range(NCORES):
        in_maps.append({
            "q0": q0b,
            "wq": np.ascontiguousarray(WQ[c]).astype(bf16),
            "bq": np.ascontiguousarray(bQ[c].reshape(8, 128).T),
            "wk": np.ascontiguousarray(WK[c]).astype(bf16),
            "kt": np.ascontiguousarray(K[c * MS:(c + 1) * MS].T).astype(bf16),
            "v": np.ascontiguousarray(
                V[c * MS:(c + 1) * MS].reshape(MS, PIX)).astype(bf16),
        })

    res = run_bass_kernel_spmd(nc, in_maps, list(range(NCORES))).results
    outs = np.stack([np.asarray(res[c]["out"]).reshape(128, 128, 3)
                     for c in range(NCORES)])
    return outs.astype(np.float32)


# revision 11
# speedup vs baseline: 1.0694x; 1.0694x over previous
"""Bass/Tile TRN2 kernel for nn_MultiHeadAttention_11330123727139.

Reference computation (full shapes):
  Q [1,1024], K [2048,1024], V [2048,128,128,3],
  WQ/WK [8,1024,1024], bQ/bK [8,1024]
  q = Q @ WQ[h].T + bQ[h]; k = K @ WK[h].T + bK[h]
  logits[h,m] = (q[h] . k[h,m]) / 1024
  w = softmax(logits, axis=m);  out[h] = sum_m w[h,m] * V[m]

Algebra (exact): q.k = q^T WK K[m] + q.bK; the bK term is constant in m
so it cancels in softmax -> bK unused. With t[h] = WK[h]^T q[h]:
logits[h,m] = t[h].K[m]/1024. The 34-GFLOP K-projection disappears;
the kernel is memory-bound on V.

logits/1024 ~ O(0.05) so softmax needs no max subtraction. We compute
unnormalized u = exp(l/1024), the weighted V sum with u, and carry
sum(u) inside the ReduceScatter payload; division happens after the
collective. Only 2 collectives: AllGather(t), ReduceScatter(partials).

Precision: V / W-matrices / K cast to bf16 on the host (half DMA,
1-pass bf16 matmul); fp32 accumulation everywhere. Simulated
end-to-end relative error: 2.4e-3.

Sharding (8 cores): core c owns head c's WQ/WK and K/V rows
[256c, 256c+256). ReduceScatter leaves head c's finished image on
core c; the host stacks the 8 images.
"""

import numpy as np

D = 1024
H = 8
M = 2048
NCORES = 8
MS = M // NCORES          # 256 neighbors per core
PIX = 128 * 128 * 3       # 49152 pixels per image
CH = 4096                 # V free-dim chunk per DMA tile (bf16 -> 1MB)
RSW = PIX + 8             # RS row: 49152 pixels + sumexp + 7 pad (32B align)


def _build_nc():
    import concourse.bacc as bacc
    import concourse.mybir as mybir
    import concourse.tile as tile
    from concourse.masks import make_identity
    from concourse.tile import add_dep_helper

    fp32 = mybir.dt.float32
    bf16 = mybir.dt.bfloat16
    ALU = mybir.AluOpType
    AX = mybir.AxisListType
    ACT = mybir.ActivationFunctionType

    nc = bacc.Bacc("TRN2", target_bir_lowering=False, debug=False,
                   num_devices=NCORES)

    q0 = nc.dram_tensor("q0", [1, D], bf16, kind="ExternalInput")
    wq = nc.dram_tensor("wq", [D, D], bf16, kind="ExternalInput")
    bq = nc.dram_tensor("bq", [128, 8], fp32, kind="ExternalInput")
    wk = nc.dram_tensor("wk", [D, D], bf16, kind="ExternalInput")
    kt = nc.dram_tensor("kt", [D, MS], bf16, kind="ExternalInput")
    v = nc.dram_tensor("v", [MS, PIX], bf16, kind="ExternalInput")
    out = nc.dram_tensor("out", [PIX], fp32, kind="ExternalOutput")

    RG = [list(range(NCORES))]

    with tile.TileContext(nc) as tc:
        with (
            tc.tile_pool(name="wts", bufs=1) as wts,
            tc.tile_pool(name="sm", bufs=1) as sm,
            tc.tile_pool(name="scr", bufs=2) as scr,
            tc.tile_pool(name="vp", bufs=6) as vp,
            tc.tile_pool(name="stg", bufs=3) as stg,
            tc.tile_pool(name="ps8", bufs=8, space="PSUM") as ps8,
            tc.tile_pool(name="dram", bufs=1, space="DRAM") as dram,
        ):
            ident = sm.tile([128, 128], fp32)
            make_identity(nc, ident)

            # Dummy collective with no data deps: absorbs the one-time
            # runtime collective-entry barrier + ncfw warmup during the
            # setup phase, so AllGather(t) later pays only mesh latency.
            warm_in = dram.tile([8], fp32)
            warm_out = dram.tile([64], fp32, addr_space="Shared")
            nc.gpsimd.collective_compute(
                "AllGather", ALU.bypass, replica_groups=RG,
                ins=[warm_in[:]], outs=[warm_out[:]])

            # ---- load inputs ----
            qb = wts.tile([128, D], bf16)
            nc.sync.dma_start(qb[:], q0[0:1, :].to_broadcast([128, D]))

            # wq/wk gate t (and thus the AllGather) -> sync queue, first.
            # kt is only needed post-AG -> scalar queue, ahead of v1 loads.
            wq_sb = wts.tile([128, 8 * D], bf16)
            wk_sb = wts.tile([128, 8 * D], bf16)
            kt_sb = wts.tile([128, 8 * MS], bf16)
            for i in range(8):
                nc.sync.dma_start(wq_sb[:, i * D:(i + 1) * D],
                                  wq[i * 128:(i + 1) * 128, :])
                nc.sync.dma_start(wk_sb[:, i * D:(i + 1) * D],
                                  wk[i * 128:(i + 1) * 128, :])
                nc.scalar.dma_start(kt_sb[:, i * MS:(i + 1) * MS],
                                    kt[i * 128:(i + 1) * 128, :])
            bq_sb = sm.tile([128, 8], fp32)
            nc.sync.dma_start(bq_sb[:], bq[:, :])

            # ---- q[e] = sum_d WQ[e,d]*Q[d] + bQ[e] ----
            q_cols = sm.tile([128, 8], fp32)
            for i in range(8):
                sc = scr.tile([128, D], bf16, tag="ttr", name=f"sc{i}")
                nc.vector.tensor_tensor(out=sc[:],
                                        in0=wq_sb[:, i * D:(i + 1) * D],
                                        in1=qb[:], op=ALU.mult)
                nc.vector.tensor_reduce(out=q_cols[:, i:i + 1], in_=sc[:],
                                        axis=AX.X, op=ALU.add)
            nc.vector.tensor_tensor(out=q_cols[:], in0=q_cols[:],
                                    in1=bq_sb[:], op=ALU.add)
            q_colsb = sm.tile([128, 8], bf16)
            nc.vector.tensor_copy(out=q_colsb[:], in_=q_cols[:])

            # ---- t[d] = sum_e WK[e,d] q[e] -> t_row [1,1024] fp32 ----
            t_row = sm.tile([1, D], fp32)
            for j in range(2):
                t_ps = ps8.tile([1, 512], fp32, tag="ps", name=f"tps{j}")
                for i in range(8):
                    nc.tensor.matmul(
                        t_ps[:],
                        lhsT=q_colsb[:, i:i + 1],
                        rhs=wk_sb[:, i * D + j * 512: i * D + (j + 1) * 512],
                        start=(i == 0),
                        stop=(i == 7),
                    )
                nc.vector.tensor_copy(out=t_row[0:1, j * 512:(j + 1) * 512],
                                      in_=t_ps[:])

            # ---- AllGather t (the only pre-V collective) ----
            t_loc = dram.tile([D], fp32)
            nc.sync.dma_start(t_loc[:], t_row[0:1, :])
            t_all = dram.tile([H, D], fp32, addr_space="Shared")
            nc.gpsimd.collective_compute(
                "AllGather", ALU.bypass, replica_groups=RG,
                ins=[t_loc[:]], outs=[t_all[:]])

            # t_all -> bf16 stationary tiles [128 d, 8 h] per d-tile
            t_nat = sm.tile([8, D], fp32)
            nc.sync.dma_start(t_nat[:], t_all[:])
            t_stat = sm.tile([128, 64], bf16)
            for j in range(8):
                tp = ps8.tile([128, 8], fp32, tag="ps", name=f"ttp{j}")
                nc.tensor.transpose(tp[:], t_nat[:, j * 128:(j + 1) * 128],
                                    ident[0:8, 0:8])
                nc.vector.tensor_copy(out=t_stat[:, j * 8:(j + 1) * 8],
                                      in_=tp[:])

            # ---- local logits [8 heads, 256 m] (fp32 accum) ----
            lg_ps = ps8.tile([8, MS], fp32, tag="ps", name="lgps")
            for j in range(8):
                nc.tensor.matmul(
                    lg_ps[:],
                    lhsT=t_stat[:, j * 8:(j + 1) * 8],
                    rhs=kt_sb[:, j * MS:(j + 1) * MS],
                    start=(j == 0),
                    stop=(j == 7),
                )
            lg_loc_sb = sm.tile([8, MS], fp32)
            nc.vector.tensor_copy(out=lg_loc_sb[:], in_=lg_ps[:])

            # ---- unnormalized weights u = exp(l/1024), se = sum(u) ----
            w_exp = sm.tile([8, MS], fp32)
            se = sm.tile([8, 1], fp32)
            nc.scalar.activation(out=w_exp[:], in_=lg_loc_sb[:], func=ACT.Exp,
                                 scale=1.0 / float(D), accum_out=se[:])

            wvb = sm.tile([128, 16], bf16)
            for k in range(2):
                wp = ps8.tile([128, 8], fp32, tag="ps", name=f"wp{k}")
                nc.tensor.transpose(wp[:], w_exp[:, k * 128:(k + 1) * 128],
                                    ident[0:8, 0:8])
                nc.vector.tensor_copy(out=wvb[:, k * 8:(k + 1) * 8], in_=wp[:])

            # ---- streamed weighted sum over V shard (bf16 x bf16 -> fp32).
            # Two ReduceScatters so the first overlaps the V-loop tail;
            # sumexp rides in payload A (slot PIXH) so half-A normalization
            # can start as soon as RS#1 lands. ----
            PIXH = PIX // 2
            NBLK = PIX // CH
            rs_in_a = dram.tile([H, PIXH + 8], fp32)
            rs_in_b = dram.tile([H, PIXH], fp32)
            nc.sync.dma_start(rs_in_a[:, PIXH:PIXH + 1], se[:])
            zpad = sm.tile([8, 7], fp32)
            nc.vector.memset(zpad[:], 0.0)
            nc.sync.dma_start(rs_in_a[:, PIXH + 1:PIXH + 8], zpad[:])

            rs_out_a = dram.tile([PIXH + 8], fp32)
            rs_out_b = dram.tile([PIXH], fp32)

            last_v0 = last_vcopy = last_scopy = None
            for blk in range(NBLK):
                v0 = vp.tile([128, CH], bf16, tag="v0", name=f"v0_{blk}")
                v1 = vp.tile([128, CH], bf16, tag="v1", name=f"v1_{blk}")
                last_v0 = nc.sync.dma_start(v0[:],
                                            v[0:128, blk * CH:(blk + 1) * CH])
                nc.scalar.dma_start(v1[:], v[128:256, blk * CH:(blk + 1) * CH])
                for g in range(CH // 2048):
                    pss = []
                    for s in range(4):
                        ps_t = ps8.tile([8, 512], fp32, tag="ps",
                                        name=f"ps{blk}_{g}_{s}")
                        pss.append(ps_t)
                    for s in range(4):
                        o = g * 2048 + s * 512
                        nc.tensor.matmul(pss[s][:], lhsT=wvb[:, 0:8],
                                         rhs=v0[:, o:o + 512],
                                         start=True, stop=False)
                    for s in range(4):
                        o = g * 2048 + s * 512
                        nc.tensor.matmul(pss[s][:], lhsT=wvb[:, 8:16],
                                         rhs=v1[:, o:o + 512],
                                         start=False, stop=True)
                    st = stg.tile([8, 2048], fp32, tag="st",
                                  name=f"st{blk}_{g}")
                    for s in range(4):
                        if s % 2 == 0:
                            last_scopy = nc.scalar.copy(
                                out=st[:, s * 512:(s + 1) * 512],
                                in_=pss[s][:])
                        else:
                            last_vcopy = nc.vector.tensor_copy(
                                out=st[:, s * 512:(s + 1) * 512],
                                in_=pss[s][:])
                    o = blk * CH + g * 2048
                    if o < PIXH:
                        nc.gpsimd.dma_start(rs_in_a[:, o:o + 2048], st[:])
                    else:
                        nc.gpsimd.dma_start(rs_in_b[:, o - PIXH:o - PIXH + 2048],
                                            st[:])
                if blk == NBLK // 2 - 1:
                    nc.gpsimd.collective_compute(
                        "ReduceScatter", ALU.add, replica_groups=RG,
                        ins=[rs_in_a[:]], outs=[rs_out_a[:]])

            nc.gpsimd.collective_compute(
                "ReduceScatter", ALU.add, replica_groups=RG,
                ins=[rs_in_b[:]], outs=[rs_out_b[:]])

            # normalize: out = rs_out / sum(u); half A overlaps RS#2.
            # Explicit ordering deps keep these gated ops BEHIND the V-loop
            # work in each in-order engine queue (no priority inversion).
            def after(inst, anchor):
                if anchor is not None:
                    add_dep_helper(inst.ins, anchor.ins, False,
                                   "normalize ordered after V loop")
                return inst

            seb = sm.tile([128, 1], fp32)
            after(nc.sync.dma_start(
                seb[:], rs_out_a[PIXH:PIXH + 1].to_broadcast([128, 1])),
                last_v0)
            inv = sm.tile([128, 1], fp32)
            after(nc.vector.reciprocal(inv[:], seb[:]), last_vcopy)
            ob_a = sm.tile([128, PIXH // 128], fp32)
            after(nc.sync.dma_start(ob_a[:], rs_out_a[0:PIXH]), last_v0)
            nc.vector.tensor_scalar_mul(ob_a[:], ob_a[:], inv[:])
            nc.sync.dma_start(out[0:PIXH], ob_a[:])
            ob_b = sm.tile([128, PIXH // 128], fp32)
            after(nc.scalar.dma_start(ob_b[:], rs_out_b[0:PIXH]), last_scopy)
            nc.vector.tensor_scalar_mul(ob_b[:], ob_b[:], inv[:])
            nc.scalar.dma_start(out[PIXH:PIX], ob_b[:])

    nc.compile()
    return nc


_NC_CACHE = []


def kernel(Q, K, V, WQ, bQ, WK, bK):
    import ml_dtypes
    from concourse.bass_utils import run_bass_kernel_spmd

    if not _NC_CACHE:
        _NC_CACHE.append(_build_nc())
    nc = _NC_CACHE[0]

    bf16 = ml_dtypes.bfloat16
    Q = np.asarray(Q, dtype=np.float32)
    K = np.asarray(K, dtype=np.float32)
    V = np.asarray(V, dtype=np.float32)
    WQ = np.asarray(WQ, dtype=np.float32)
    bQ = np.asarray(bQ, dtype=np.float32)
    WK = np.asarray(WK, dtype=np.float32)

    q0b = np.ascontiguousarray(Q.reshape(1, D)).astype(bf16)
    in_maps = []
    for c in range(NCORES):
        in_maps.append({
            "q0": q0b,
            "wq": np.ascontiguousarray(WQ[c]).astype(bf16),
            "bq": np.ascontiguousarray(bQ[c].reshape(8, 128).T),
            "wk": np.ascontiguousarray(WK[c]).astype(bf16),
            "kt": np.ascontiguousarray(K[c * MS:(c + 1) * MS].T).astype(bf16),
            "v": np.ascontiguousarray(
                V[c * MS:(c + 1) * MS].reshape(MS, PIX)).astype(bf16),
        })

    res = run_bass_kernel_spmd(nc, in_maps, list(range(NCORES))).results
    outs = np.stack([np.asarray(res[c]["out"]).reshape(128, 128, 3)
                     for c in range(NCORES)])
    return outs.astype(np.float32)


# revision 12
# speedup vs baseline: 1.1452x; 1.0709x over previous
"""Bass/Tile TRN2 kernel for nn_MultiHeadAttention_11330123727139.

Reference computation (full shapes):
  Q [1,1024], K [2048,1024], V [2048,128,128,3],
  WQ/WK [8,1024,1024], bQ/bK [8,1024]
  q = Q @ WQ[h].T + bQ[h]; k = K @ WK[h].T + bK[h]
  logits[h,m] = (q[h] . k[h,m]) / 1024
  w = softmax(logits, axis=m);  out[h] = sum_m w[h,m] * V[m]

Algebra (exact): q.k = q^T WK K[m] + q.bK; the bK term is constant in m
so it cancels in softmax -> bK unused. With t[h] = WK[h]^T q[h]:
logits[h,m] = t[h].K[m]/1024. The 34-GFLOP K-projection disappears;
the kernel is memory-bound on V.

logits/1024 ~ O(0.05) so softmax needs no max subtraction. We compute
unnormalized u = exp(l/1024), the weighted V sum with u, and carry
sum(u) inside the ReduceScatter payload; division happens after the
collective. Only 2 collectives: AllGather(t), ReduceScatter(partials).

Precision: V / W-matrices / K cast to bf16 on the host (half DMA,
1-pass bf16 matmul); fp32 accumulation everywhere. Simulated
end-to-end relative error: 2.4e-3.

Sharding (8 cores): core c owns head c's WQ/WK and K/V rows
[256c, 256c+256). ReduceScatter leaves head c's finished image on
core c; the host stacks the 8 images.
"""

import numpy as np

D = 1024
H = 8
M = 2048
NCORES = 8
MS = M // NCORES          # 256 neighbors per core
PIX = 128 * 128 * 3       # 49152 pixels per image
CH = 4096                 # V free-dim chunk per DMA tile (bf16 -> 1MB)
RSW = PIX + 8             # RS row: 49152 pixels + sumexp + 7 pad (32B align)


def _build_nc():
    import concourse.bacc as bacc
    import concourse.mybir as mybir
    import concourse.tile as tile
    from concourse.masks import make_identity
    from concourse.tile import add_dep_helper

    fp32 = mybir.dt.float32
    bf16 = mybir.dt.bfloat16
    ALU = mybir.AluOpType
    AX = mybir.AxisListType
    ACT = mybir.ActivationFunctionType

    nc = bacc.Bacc("TRN2", target_bir_lowering=False, debug=False,
                   num_devices=NCORES)

    q0 = nc.dram_tensor("q0", [1, D], bf16, kind="ExternalInput")
    wq = nc.dram_tensor("wq", [D, D], bf16, kind="ExternalInput")
    bq = nc.dram_tensor("bq", [128, 8], fp32, kind="ExternalInput")
    wk = nc.dram_tensor("wk", [D, D], bf16, kind="ExternalInput")
    kt = nc.dram_tensor("kt", [D, MS], bf16, kind="ExternalInput")
    v = nc.dram_tensor("v", [MS, PIX], bf16, kind="ExternalInput")
    out = nc.dram_tensor("out", [PIX], fp32, kind="ExternalOutput")

    RG = [list(range(NCORES))]

    with tile.TileContext(nc) as tc:
        with (
            tc.tile_pool(name="wts", bufs=1) as wts,
            tc.tile_pool(name="sm", bufs=1) as sm,
            tc.tile_pool(name="scr", bufs=2) as scr,
            tc.tile_pool(name="vp", bufs=7) as vp,
            tc.tile_pool(name="stg", bufs=4) as stg,
            tc.tile_pool(name="ps8", bufs=8, space="PSUM") as ps8,
            tc.tile_pool(name="dram", bufs=1, space="DRAM") as dram,
        ):
            ident = sm.tile([128, 128], fp32)
            make_identity(nc, ident)

            # ---- load inputs ----
            qb = wts.tile([128, D], bf16)
            nc.sync.dma_start(qb[:], q0[0:1, :].to_broadcast([128, D]))

            # wq/wk gate t (and thus the AllGather) -> sync queue, first.
            # kt is only needed post-AG -> scalar queue, ahead of v1 loads.
            wq_sb = wts.tile([128, 8 * D], bf16)
            wk_sb = wts.tile([128, 8 * D], bf16)
            kt_sb = wts.tile([128, 8 * MS], bf16)
            for i in range(8):
                nc.sync.dma_start(wq_sb[:, i * D:(i + 1) * D],
                                  wq[i * 128:(i + 1) * 128, :])
                nc.sync.dma_start(wk_sb[:, i * D:(i + 1) * D],
                                  wk[i * 128:(i + 1) * 128, :])
                nc.scalar.dma_start(kt_sb[:, i * MS:(i + 1) * MS],
                                    kt[i * 128:(i + 1) * 128, :])
            bq_sb = sm.tile([128, 8], fp32)
            nc.sync.dma_start(bq_sb[:], bq[:, :])

            # ---- q[e] = sum_d WQ[e,d]*Q[d] + bQ[e] ----
            q_cols = sm.tile([128, 8], fp32)
            for i in range(8):
                sc = scr.tile([128, D], bf16, tag="ttr", name=f"sc{i}")
                nc.vector.tensor_tensor(out=sc[:],
                                        in0=wq_sb[:, i * D:(i + 1) * D],
                                        in1=qb[:], op=ALU.mult)
                nc.vector.tensor_reduce(out=q_cols[:, i:i + 1], in_=sc[:],
                                        axis=AX.X, op=ALU.add)
            nc.vector.tensor_tensor(out=q_cols[:], in0=q_cols[:],
                                    in1=bq_sb[:], op=ALU.add)
            q_colsb = sm.tile([128, 8], bf16)
            nc.vector.tensor_copy(out=q_colsb[:], in_=q_cols[:])

            # ---- t[d] = sum_e WK[e,d] q[e] -> t_row [1,1024] fp32 ----
            t_row = sm.tile([1, D], fp32)
            for j in range(2):
                t_ps = ps8.tile([1, 512], fp32, tag="ps", name=f"tps{j}")
                for i in range(8):
                    nc.tensor.matmul(
                        t_ps[:],
                        lhsT=q_colsb[:, i:i + 1],
                        rhs=wk_sb[:, i * D + j * 512: i * D + (j + 1) * 512],
                        start=(i == 0),
                        stop=(i == 7),
                    )
                nc.vector.tensor_copy(out=t_row[0:1, j * 512:(j + 1) * 512],
                                      in_=t_ps[:])

            # ---- AllGather t (the only pre-V collective) ----
            t_loc = dram.tile([D], fp32)
            nc.sync.dma_start(t_loc[:], t_row[0:1, :])
            t_all = dram.tile([H, D], fp32, addr_space="Shared")
            nc.gpsimd.collective_compute(
                "AllGather", ALU.bypass, replica_groups=RG,
                ins=[t_loc[:]], outs=[t_all[:]])

            # t_all -> bf16 stationary tiles [128 d, 8 h] per d-tile
            t_nat = sm.tile([8, D], fp32)
            nc.sync.dma_start(t_nat[:], t_all[:])
            t_stat = sm.tile([128, 64], bf16)
            for j in range(8):
                tp = ps8.tile([128, 8], fp32, tag="ps", name=f"ttp{j}")
                nc.tensor.transpose(tp[:], t_nat[:, j * 128:(j + 1) * 128],
                                    ident[0:8, 0:8])
                nc.vector.tensor_copy(out=t_stat[:, j * 8:(j + 1) * 8],
                                      in_=tp[:])

            # ---- local logits [8 heads, 256 m] (fp32 accum) ----
            lg_ps = ps8.tile([8, MS], fp32, tag="ps", name="lgps")
            for j in range(8):
                nc.tensor.matmul(
                    lg_ps[:],
                    lhsT=t_stat[:, j * 8:(j + 1) * 8],
                    rhs=kt_sb[:, j * MS:(j + 1) * MS],
                    start=(j == 0),
                    stop=(j == 7),
                )
            lg_loc_sb = sm.tile([8, MS], fp32)
            nc.vector.tensor_copy(out=lg_loc_sb[:], in_=lg_ps[:])

            # ---- unnormalized weights u = exp(l/1024), se = sum(u) ----
            w_exp = sm.tile([8, MS], fp32)
            se = sm.tile([8, 1], fp32)
            nc.scalar.activation(out=w_exp[:], in_=lg_loc_sb[:], func=ACT.Exp,
                                 scale=1.0 / float(D), accum_out=se[:])

            wvb = sm.tile([128, 16], bf16)
            for k in range(2):
                wp = ps8.tile([128, 8], fp32, tag="ps", name=f"wp{k}")
                nc.tensor.transpose(wp[:], w_exp[:, k * 128:(k + 1) * 128],
                                    ident[0:8, 0:8])
                nc.vector.tensor_copy(out=wvb[:, k * 8:(k + 1) * 8], in_=wp[:])

            # ---- streamed weighted sum over V shard (bf16 x bf16 -> fp32).
            # Two ReduceScatters so the first overlaps the V-loop tail;
            # sumexp rides in payload A (slot PIXH) so half-A normalization
            # can start as soon as RS#1 lands. ----
            PIXH = PIX // 2
            NBLK = PIX // CH
            rs_in_a = dram.tile([H, PIXH + 8], fp32)
            rs_in_b = dram.tile([H, PIXH], fp32)
            nc.sync.dma_start(rs_in_a[:, PIXH:PIXH + 1], se[:])
            zpad = sm.tile([8, 7], fp32)
            nc.vector.memset(zpad[:], 0.0)
            nc.sync.dma_start(rs_in_a[:, PIXH + 1:PIXH + 8], zpad[:])

            rs_out_a = dram.tile([PIXH + 8], fp32)
            rs_out_b = dram.tile([PIXH], fp32)

            last_v0 = last_vcopy = last_scopy = None
            for blk in range(NBLK):
                v0 = vp.tile([128, CH], bf16, tag="v0", name=f"v0_{blk}")
                v1 = vp.tile([128, CH], bf16, tag="v1", name=f"v1_{blk}")
                last_v0 = nc.sync.dma_start(v0[:],
                                            v[0:128, blk * CH:(blk + 1) * CH])
                nc.scalar.dma_start(v1[:], v[128:256, blk * CH:(blk + 1) * CH])
                for g in range(CH // 2048):
                    pss = []
                    for s in range(4):
                        ps_t = ps8.tile([8, 512], fp32, tag="ps",
                                        name=f"ps{blk}_{g}_{s}")
                        pss.append(ps_t)
                    for s in range(4):
                        o = g * 2048 + s * 512
                        nc.tensor.matmul(pss[s][:], lhsT=wvb[:, 0:8],
                                         rhs=v0[:, o:o + 512],
                                         start=True, stop=False)
                    for s in range(4):
                        o = g * 2048 + s * 512
                        nc.tensor.matmul(pss[s][:], lhsT=wvb[:, 8:16],
                                         rhs=v1[:, o:o + 512],
                                         start=False, stop=True)
                    st = stg.tile([8, 2048], fp32, tag="st",
                                  name=f"st{blk}_{g}")
                    for s in range(4):
                        if s % 2 == 0:
                            last_scopy = nc.scalar.copy(
                                out=st[:, s * 512:(s + 1) * 512],
                                in_=pss[s][:])
                        else:
                            last_vcopy = nc.vector.tensor_copy(
                                out=st[:, s * 512:(s + 1) * 512],
                                in_=pss[s][:])
                    o = blk * CH + g * 2048
                    if o < PIXH:
                        nc.gpsimd.dma_start(rs_in_a[:, o:o + 2048], st[:])
                    else:
                        nc.gpsimd.dma_start(rs_in_b[:, o - PIXH:o - PIXH + 2048],
                                            st[:])
                if blk == NBLK // 2 - 1:
                    nc.gpsimd.collective_compute(
                        "ReduceScatter", ALU.add, replica_groups=RG,
                        ins=[rs_in_a[:]], outs=[rs_out_a[:]])

            nc.gpsimd.collective_compute(
                "ReduceScatter", ALU.add, replica_groups=RG,
                ins=[rs_in_b[:]], outs=[rs_out_b[:]])

            # normalize: out = rs_out / sum(u); half A overlaps RS#2.
            # Explicit ordering deps keep these gated ops BEHIND the V-loop
            # work in each in-order engine queue (no priority inversion).
            def after(inst, anchor):
                if anchor is not None:
                    add_dep_helper(inst.ins, anchor.ins, False,
                                   "normalize ordered after V loop")
                return inst

            seb = sm.tile([128, 1], fp32)
            after(nc.sync.dma_start(
                seb[:], rs_out_a[PIXH:PIXH + 1].to_broadcast([128, 1])),
                last_v0)
            inv = sm.tile([128, 1], fp32)
            after(nc.vector.reciprocal(inv[:], seb[:]), last_vcopy)
            ob_a = sm.tile([128, PIXH // 128], fp32)
            after(nc.sync.dma_start(ob_a[:], rs_out_a[0:PIXH]), last_v0)
            nc.vector.tensor_scalar_mul(ob_a[:], ob_a[:], inv[:])
            nc.sync.dma_start(out[0:PIXH], ob_a[:])
            ob_b = sm.tile([128, PIXH // 128], fp32)
            after(nc.scalar.dma_start(ob_b[:], rs_out_b[0:PIXH]), last_scopy)
            nc.vector.tensor_scalar_mul(ob_b[:], ob_b[:], inv[:])
            nc.scalar.dma_start(out[PIXH:PIX], ob_b[:])

    nc.compile()
    return nc


_NC_CACHE = []


def kernel(Q, K, V, WQ, bQ, WK, bK):
    import ml_dtypes
    from concourse.bass_utils import run_bass_kernel_spmd

    if not _NC_CACHE:
        _NC_CACHE.append(_build_nc())
    nc = _NC_CACHE[0]

    bf16 = ml_dtypes.bfloat16
    Q = np.asarray(Q, dtype=np.float32)
    K = np.asarray(K, dtype=np.float32)
    V = np.asarray(V, dtype=np.float32)
    WQ = np.asarray(WQ, dtype=np.float32)
    bQ = np.asarray(bQ, dtype=np.float32)
    WK = np.asarray(WK, dtype=np.float32)

    q0b = np.ascontiguousarray(Q.reshape(1, D)).astype(bf16)
    in_maps = []
    for c in range(NCORES):
        in_maps.append({
            "q0": q0b,
            "wq": np.ascontiguousarray(WQ[c]).astype(bf16),
            "bq": np.ascontiguousarray(bQ[c].reshape(8, 128).T),
            "wk": np.ascontiguousarray(WK[c]).astype(bf16),
            "kt": np.ascontiguousarray(K[c * MS:(c + 1) * MS].T).astype(bf16),
            "v": np.ascontiguousarray(
                V[c * MS:(c + 1) * MS].reshape(MS, PIX)).astype(bf16),
        })

    res = run_bass_kernel_spmd(nc, in_maps, list(range(NCORES))).results
    outs = np.stack([np.asarray(res[c]["out"]).reshape(128, 128, 3)
                     for c in range(NCORES)])
    return outs.astype(np.float32)
